# revision 3
# baseline (speedup 1.0000x reference)
"""TRN2 Bass kernel v3 for nn_Knowledge_Base (retrieval_knn).

reference:
    proj = word_output @ W.T + b            # [B,S,H]
    dis  = -sqrt(sum((proj[...,None,:] - op_emb)**2, -1))   # [B,S,O]
    prob = softmax(dis, -1); prob[prob < 0.3] = 0

Strategy (8 cores data-parallel, 1024 tokens/core, 2 tiles of TT=512):
  Shift the codebook by the bias (d2 = ||q - (e-b)||^2, q = x@W.T) so the
  bias matmul disappears. Then per the margin analysis (two probs sit
  3.3e-5 from the 0.3 threshold) each d2 term gets just enough precision:
  - dot term -2 q.e: computed EXACTLY from x via host-precomputed
    A = -2(e-b)@W [E,O]: split3 bf16 on (A, x) with x = xh+xl an exact
    bf16 pair. m<=64 stationaries make this cheap (12 chunk-matmuls).
  - mm1 (q, for the norm term only): ONE fp32r matmul chain. HW rounds
    operands to 11-bit mantissa (round-to-nearest, probe-validated) and
    runs at full bf16 rate for free dim >= 256 - 3x cheaper than the
    split3-bf16 scheme the norm's error budget would otherwise force.
  - ||q||^2: ACT squares q (fp32), DVE splits to bf16 hi/lo, all-ones
    [128,32] stationaries accumulate into the dot PSUM (m=32 rows share
    the stream cost).
  - ||e-b||^2 enters as the ACT bias operand of Ln (free).
  - softmax: u=Ln(d2), transpose u to [t,o] on PE, s=exp(0.5u)=sqrt(d2),
    p=exp(-s); threshold as (p >= 0.3 sum) mask multiply.
  Host-simulated end-to-end: rel err 1.0e-05, prob error at the
  near-threshold elements <= 0.15 of their margin.
"""
import sys
sys.path.insert(0, "/opt/trn_rl_repo")
import numpy as np
import ml_dtypes

import concourse.bacc as bacc
import concourse.tile as tile
from concourse import mybir
from concourse import bass_utils

BF = ml_dtypes.bfloat16
P = 128
B, S, E, H, O = 4, 2048, 768, 512, 32
NCORES = 8
TOK = B * S                  # 8192
TPC = TOK // NCORES          # 1024 tokens per core
TT = 512                     # t-tile size
NTT = TPC // TT              # 2 t-tiles per core
EC = E // P                  # 6 e-chunks
HC = H // P                  # 4 h-chunks
NC_ = TT // P                # 4 column blocks per tile
THRESH = 0.3

_CACHE = {}


def _split(a):
    hi = np.asarray(a, np.float32).astype(BF)
    lo = (np.asarray(a, np.float32) - hi.astype(np.float32)).astype(BF)
    return hi, lo


def _build(n_reps=1):
    nc = bacc.Bacc("TRN2", target_bir_lowering=False, debug=False,
                   num_devices=NCORES)
    dt = mybir.dt
    xh_d = nc.dram_tensor("xh", [E, TPC], dt.bfloat16, kind="ExternalInput").ap()
    xl_d = nc.dram_tensor("xl", [E, TPC], dt.bfloat16, kind="ExternalInput").ap()
    wt_d = nc.dram_tensor("wt", [E, H], dt.float32r, kind="ExternalInput").ap()
    # ahl: [E, 64] = [Ah | Al] bf16
    ahl_d = nc.dram_tensor("ahl", [E, 2 * O], dt.bfloat16, kind="ExternalInput").ap()
    # nrme: [P, O] f32, ||e_o - b||^2 replicated across partitions
    nrme_d = nc.dram_tensor("nrme", [P, O], dt.float32, kind="ExternalInput").ap()
    out_d = nc.dram_tensor("out", [TPC, O], dt.float32, kind="ExternalOutput").ap()

    AF = mybir.ActivationFunctionType
    ALU = mybir.AluOpType

    with tile.TileContext(nc) as tc:
        with tc.tile_pool(name="consts", bufs=1) as consts, \
             tc.tile_pool(name="xin", bufs=2) as xin, \
             tc.tile_pool(name="work", bufs=2) as work, \
             tc.tile_pool(name="psq", bufs=1, space="PSUM") as psq, \
             tc.tile_pool(name="psd", bufs=1, space="PSUM") as psd, \
             tc.tile_pool(name="pst", bufs=1, space="PSUM") as pst:

            # ---- constants ----
            wt_sb = []
            for e in range(EC):
                w_e = consts.tile([P, H], dt.float32r, tag=f"wt{e}")
                nc.sync.dma_start(w_e, wt_d[e * P:(e + 1) * P, :])
                wt_sb.append(w_e)
            ahl_sb = consts.tile([P, EC, 2 * O], dt.bfloat16)
            nc.sync.dma_start(ahl_sb, ahl_d.rearrange("(c p) o -> p c o", p=P))
            nrme_sb = consts.tile([P, O], dt.float32)
            nc.sync.dma_start(nrme_sb, nrme_d)
            ones_sb = consts.tile([P, O], dt.bfloat16)
            nc.vector.memset(ones_sb, 1.0)
            from concourse.masks import make_identity
            ident_sb = consts.tile([P, P], dt.float32)
            make_identity(nc, ident_sb)

            for rep in range(n_reps):
                # ---- stage x for both tiles; reconstruct fp32 x ----
                xh_sb = {}
                xl_sb = {}
                x32_sb = {}
                for t in range(NTT):
                    tsl = slice(t * TT, (t + 1) * TT)
                    for e in range(EC):
                        esl = slice(e * P, (e + 1) * P)
                        xh_e = xin.tile([P, TT], dt.bfloat16, tag=f"xh{t}{e}")
                        xl_e = xin.tile([P, TT], dt.bfloat16, tag=f"xl{t}{e}")
                        nc.sync.dma_start(xh_e, xh_d[esl, tsl])
                        nc.sync.dma_start(xl_e, xl_d[esl, tsl])
                        x32_e = xin.tile([P, TT], dt.float32r, tag=f"x32{t}{e}")
                        nc.vector.tensor_tensor(x32_e, xh_e, xl_e, ALU.add)
                        xh_sb[t, e] = xh_e
                        xl_sb[t, e] = xl_e
                        x32_sb[t, e] = x32_e

                for t in range(NTT):
                    tsl = slice(t * TT, (t + 1) * TT)

                    # ---- mm1: q = x @ W.T in fp32r, e-outer/h-inner ----
                    ps_h = [psq.tile([P, TT], dt.float32, tag=f"psq{h}",
                                     name=f"psq{h}")
                            for h in range(HC)]
                    for e in range(EC):
                        mv = x32_sb[t, e]
                        for h in range(HC):
                            hsl = slice(h * P, (h + 1) * P)
                            nc.tensor.matmul(ps_h[h], wt_sb[e][:, hsl], mv,
                                             start=(e == 0), stop=(e == EC - 1))

                    # squares: sq32 fp32 (ACT), split to bf16 hi/lo (DVE)
                    sqh_sb = []
                    sql_sb = []
                    for h in range(HC):
                        sq32 = work.tile([P, TT], dt.float32, tag=f"sq32_{h}")
                        nc.scalar.activation(sq32, ps_h[h], AF.Square)
                        sqh = work.tile([P, TT], dt.bfloat16, tag=f"sqh{h}")
                        sql = work.tile([P, TT], dt.bfloat16, tag=f"sql{h}")
                        nc.scalar.copy(sqh, sq32)
                        nc.vector.tensor_tensor(sql, sq32, sqh, ALU.subtract)
                        sqh_sb.append(sqh)
                        sql_sb.append(sql)

                    # ---- dot (split3 on A,x) + norm into PSUM ----
                    ps_dh = psd.tile([2 * O, TT], dt.float32, tag="psdh")
                    for e in range(EC):
                        nc.tensor.matmul(ps_dh, ahl_sb[:, e, :], xh_sb[t, e],
                                         start=(e == 0), stop=(e == EC - 1))
                    ps_dl = psd.tile([O, TT], dt.float32, tag="psdl")
                    k = 0
                    nmm = EC + 2 * HC
                    for e in range(EC):
                        nc.tensor.matmul(ps_dl, ahl_sb[:, e, 0:O], xl_sb[t, e],
                                         start=(k == 0), stop=(k == nmm - 1))
                        k += 1
                    for h in range(HC):
                        nc.tensor.matmul(ps_dl, ones_sb, sqh_sb[h],
                                         start=False, stop=(k == nmm - 1))
                        k += 1
                        nc.tensor.matmul(ps_dl, ones_sb, sql_sb[h],
                                         start=False, stop=(k == nmm - 1))
                        k += 1

                    # ---- stage dot PSUM to SBUF, transpose to [t, o] ----
                    dc = work.tile([2 * O, TT], dt.float32, tag="dc")
                    nc.scalar.copy(dc, ps_dh)
                    dlc = work.tile([O, TT], dt.float32, tag="dlc")
                    nc.scalar.copy(dlc, ps_dl)
                    ps_t = pst.tile([P, NC_, 3 * O], dt.float32, tag="psT")
                    for c in range(NC_):
                        csl = slice(c * P, (c + 1) * P)
                        nc.tensor.matmul(ps_t[:, c, 0:2 * O], dc[:, csl],
                                         ident_sb[:2 * O, :2 * O],
                                         is_transpose=True, start=True, stop=True)
                        nc.tensor.matmul(ps_t[:, c, 2 * O:3 * O], dlc[:, csl],
                                         ident_sb[:O, :O],
                                         is_transpose=True, start=True, stop=True)

                    # ---- d2 = hiT + loT + dlT + ||e||^2 (one PSUM input per op)
                    d2a = work.tile([P, NC_, O], dt.float32, tag="d2a")
                    nc.vector.tensor_tensor(
                        d2a, ps_t[:, :, 0:O],
                        nrme_sb[:, None, :].to_broadcast((P, NC_, O)), ALU.add)
                    d2b = work.tile([P, NC_, O], dt.float32, tag="d2b")
                    nc.vector.tensor_tensor(d2b, d2a, ps_t[:, :, O:2 * O], ALU.add)
                    d2t = work.tile([P, NC_, O], dt.float32, tag="d2t")
                    nc.vector.tensor_tensor(d2t, d2b, ps_t[:, :, 2 * O:3 * O],
                                            ALU.add)

                    # ---- softmax(-sqrt(d2)) + threshold ----
                    # s0 = exp(0.5 ln d2) ~ sqrt(d2) has ~1e-5 rel table
                    # error -> 3e-4 abs on s~32, enough to flip the razor
                    # prob. One Newton step (2s1 = s0 + d2/s0) kills it; the
                    # final exp absorbs the 0.5 as its scale.
                    u_sb = work.tile([P, NC_, O], dt.float32, tag="u")
                    nc.scalar.activation(u_sb, d2t, AF.Ln)
                    s_sb = work.tile([P, NC_, O], dt.float32, tag="s")
                    nc.scalar.activation(s_sb, u_sb, AF.Exp, scale=0.5)
                    rs = work.tile([P, NC_, O], dt.float32, tag="rs")
                    nc.vector.reciprocal(rs, s_sb)
                    dq = work.tile([P, NC_, O], dt.float32, tag="dq")
                    nc.vector.tensor_tensor(dq, d2t, rs, ALU.mult)
                    s2 = work.tile([P, NC_, O], dt.float32, tag="s2")
                    nc.vector.tensor_tensor(s2, s_sb, dq, ALU.add)
                    e_sb = work.tile([P, NC_, O], dt.float32, tag="e")
                    nc.scalar.activation(e_sb, s2, AF.Exp, scale=-0.5)
                    ssum = work.tile([P, NC_], dt.float32, tag="ssum")
                    nc.vector.reduce_sum(ssum, e_sb, axis=mybir.AxisListType.X)
                    rec = work.tile([P, NC_], dt.float32, tag="rec")
                    nc.vector.reciprocal(rec, ssum)
                    p1 = work.tile([P, NC_, O], dt.float32, tag="p1")
                    nc.vector.tensor_tensor(
                        p1, e_sb, rec[:, :, None].to_broadcast((P, NC_, O)),
                        ALU.mult)
                    msk = work.tile([P, NC_, O], dt.float32, tag="msk")
                    nc.vector.tensor_scalar(msk, p1, THRESH, None, ALU.is_ge)
                    ot = work.tile([P, NC_, O], dt.float32, tag="ot")
                    nc.vector.tensor_tensor(ot, p1, msk, ALU.mult)
                    nc.sync.dma_start(
                        out_d[tsl].rearrange("(c p) o -> p c o", p=P), ot)

    nc.compile()
    return nc


def _prep_inputs(word_output, W, b, op_emb):
    x = np.asarray(word_output, np.float32).reshape(TOK, E)
    xh, xl = _split(x)
    xth = np.ascontiguousarray(xh.T)    # [E, TOK] bf16
    xtl = np.ascontiguousarray(xl.T)

    wt = np.ascontiguousarray(np.asarray(W, np.float32).T)  # [E, H] fp32

    ep = np.asarray(op_emb, np.float64) - np.asarray(b, np.float64)  # [O, H]
    A = (-2.0 * (ep @ np.asarray(W, np.float64))).T        # [E, O] f64
    Ah, Al = _split(A.astype(np.float32))
    ahl = np.concatenate([Ah, Al], axis=1)                 # [E, 64]

    nrme = (ep * ep).sum(-1).astype(np.float32)            # [O]
    nrme = np.broadcast_to(nrme, (P, O)).copy()            # [P, O]

    common = {"wt": wt, "ahl": ahl, "nrme": nrme}
    in_maps = []
    for c in range(NCORES):
        tsl = slice(c * TPC, (c + 1) * TPC)
        m = dict(common)
        m["xh"] = np.ascontiguousarray(xth[:, tsl])
        m["xl"] = np.ascontiguousarray(xtl[:, tsl])
        in_maps.append(m)
    return in_maps


def kernel(word_output, W, b, op_emb, _trace=False):
    if "nc" not in _CACHE:
        _CACHE["nc"] = _build()
    nc = _CACHE["nc"]
    in_maps = _prep_inputs(word_output, W, b, op_emb)
    try:
        res = bass_utils.run_bass_kernel_spmd(
            nc, in_maps, core_ids=list(range(NCORES)), trace=_trace)
    except ModuleNotFoundError:
        res = bass_utils.run_bass_kernel_spmd(
            nc, in_maps, core_ids=list(range(NCORES)), trace=False)
    out = np.concatenate([r["out"] for r in res.results], axis=0)
    _CACHE["last_results"] = res
    return out.reshape(B, S, O)


if __name__ == "__main__":
    rng = np.random.default_rng(0)
    wo = rng.standard_normal((B, S, E)).astype(np.float32)
    W_ = (rng.standard_normal((H, E)) / np.sqrt(E)).astype(np.float32)
    b_ = (rng.standard_normal(H) * 0.01).astype(np.float32)
    oe = rng.standard_normal((O, H)).astype(np.float32)
    out = kernel(wo, W_, b_, oe)
    x = wo.reshape(-1, E).astype(np.float64)
    proj = x @ W_.T.astype(np.float64) + b_
    diff = proj[:, None, :] - oe
    d2 = (diff * diff).sum(-1)
    dis = -np.sqrt(d2)
    exm = np.exp(dis - dis.max(-1, keepdims=True))
    prob = exm / exm.sum(-1, keepdims=True)
    ref = np.where(prob < THRESH, 0, prob).astype(np.float32).reshape(B, S, O)
    num = np.linalg.norm(out - ref)
    den = np.linalg.norm(ref)
    print("norm rel err:", num / den)
    print("max abs err:", np.abs(out - ref).max())


# revision 4
# speedup vs baseline: 163.2362x; 163.2362x over previous
"""TRN2 Bass kernel v7 for nn_Knowledge_Base (retrieval_knn).

reference:
    proj = word_output @ W.T + b            # [B,S,H]
    dis  = -sqrt(sum((proj[...,None,:] - op_emb)**2, -1))   # [B,S,O]
    prob = softmax(dis, -1); prob[prob < 0.3] = 0

Strategy (8 cores data-parallel, 1024 tokens/core, 2 tiles of TT=512):
  Bias folded into the codebook: d2 = ||q - (e-b)||^2 with q = x@W.T.
  Two output probs sit 3.3e-5 from the 0.3 threshold, so d2 needs
  near-fp32 accuracy; fp32r matmuls (HW rounds operands to 11-bit
  mantissa, full bf16 rate at free dim >= 256) deliver exactly enough
  (host-simulated worst prob error = 0.10 of margin):
  - q: one fp32r chain per h-chunk (24 chunk-matmuls / 512-token tile)
  - dot -2 q.e: from x directly via host-precomputed A = -2(e-b)@W
    [E,O], fp32r, m=32 (6 chunk-matmuls, same moving operand as q)
  - ||q||^2: ACT squares q (fp32r out), all-ones [128,32] fp32r
    stationaries accumulate into the same [32,TT] PSUM as the dot
  - ||e-b||^2: rides the bias operand of the PSUM->SBUF copy (free)
  - sqrt via s0=exp(0.5 ln d2): ACT tables carry ~1e-5 rel error
    (3e-4 abs on s~32, enough to flip the razor probs), one DVE Newton
    step (2s = s0 + d2/s0) refines it; exp(-0.5 x) absorbs the halving.
    Elementwise work runs in the [128, 4, 32] transposed layout -
    [32, TT] tiles would use only 32 of the 128 DVE lanes.
  Scheduling: every ACT function used (square/copy/ln/exp) lives in the
  natural_log_exp_and_others table set; an explicit LoadActFuncSet pins
  it once (the automatic inserter reloads per function: 5 x 1.28 us).
  DMA order interleaves W chunks with tile-0 x chunks (serial SP queue;
  first matmul at ~1.6 us instead of ~9 us). Tile-0's softmax chain is
  emitted so it runs under tile-1's matmuls; only tile-1's chain is an
  exposed tail.
"""
import sys
sys.path.insert(0, "/opt/trn_rl_repo")
import numpy as np

import concourse.bacc as bacc
import concourse.tile as tile
from concourse import mybir
from concourse import bass_utils

P = 128
B, S, E, H, O = 4, 2048, 768, 512, 32
NCORES = 8
TOK = B * S
TPC = TOK // NCORES          # 1024 tokens per core
TT = 512
NTT = TPC // TT              # 2
EC = E // P                  # 6
HC = H // P                  # 4
NC_ = TT // P                # 4
THRESH = 0.3

_CACHE = {}


def _build(n_reps=1):
    nc = bacc.Bacc("TRN2", target_bir_lowering=False, debug=False,
                   num_devices=NCORES)
    dt = mybir.dt
    x_d = nc.dram_tensor("x", [E, TPC], dt.float32r, kind="ExternalInput").ap()
    wt_d = nc.dram_tensor("wt", [E, H], dt.float32r, kind="ExternalInput").ap()
    a22_d = nc.dram_tensor("a22", [E, O], dt.float32r, kind="ExternalInput").ap()
    ones_d = nc.dram_tensor("ones32", [P, O], dt.float32r,
                            kind="ExternalInput").ap()
    nrme_d = nc.dram_tensor("nrme", [P, O], dt.float32, kind="ExternalInput").ap()
    out_d = nc.dram_tensor("out", [TPC, O], dt.float32, kind="ExternalOutput").ap()

    AF = mybir.ActivationFunctionType
    ALU = mybir.AluOpType

    from concourse.hw_specs import get_activation_tables
    set_id = list(get_activation_tables(nc.m.arch)).index(
        "natural_log_exp_and_others")

    with tile.TileContext(nc) as tc:
        with tc.tile_pool(name="consts", bufs=1) as consts, \
             tc.tile_pool(name="xin", bufs=2) as xin, \
             tc.tile_pool(name="work", bufs=2) as work, \
             tc.tile_pool(name="psq", bufs=1, space="PSUM") as psq, \
             tc.tile_pool(name="psd", bufs=1, space="PSUM") as psd, \
             tc.tile_pool(name="pst", bufs=1, space="PSUM") as pst:

            atl = mybir.InstLoadActFuncSet(
                name=nc.get_next_instruction_name(), ins=[], outs=[])
            atl.act_func_set_id = set_id
            nc.scalar.add_instruction(atl)

            def make_x(t, e):
                x_e = xin.tile([P, TT], dt.float32r, tag=f"x{t}{e}",
                               name=f"x{t}{e}")
                tsl = slice(t * TT, (t + 1) * TT)
                nc.sync.dma_start(x_e, x_d[e * P:(e + 1) * P, tsl])
                return x_e

            # serial SP queue: interleave (wt_e, x0e) pairs; small consts
            # after pair 2 (needed from the dot onward)
            wt_sb = [None] * EC
            first_x = {}
            a22_sb = ones_sb = nrme_sb = None
            for e in range(EC):
                w_e = consts.tile([P, H], dt.float32r, tag=f"wt{e}",
                                  name=f"wt{e}")
                nc.sync.dma_start(w_e, wt_d[e * P:(e + 1) * P, :])
                wt_sb[e] = w_e
                first_x[0, e] = make_x(0, e)
                if e == 0:
                    a22_sb = consts.tile([P, EC, O], dt.float32r)
                    nc.sync.dma_start(
                        a22_sb, a22_d.rearrange("(c p) o -> p c o", p=P))
            ones_sb = consts.tile([P, O], dt.float32r)
            nc.sync.dma_start(ones_sb, ones_d)
            nrme_sb = consts.tile([P, O], dt.float32)
            nc.sync.dma_start(nrme_sb, nrme_d)
            from concourse.masks import make_identity
            ident_sb = consts.tile([P, P], dt.float32)
            make_identity(nc, ident_sb)

            for rep in range(n_reps):
                x_sb = {}
                for t in range(NTT):
                    for e in range(EC):
                        if rep == 0 and (t, e) in first_x:
                            x_sb[t, e] = first_x[t, e]
                        else:
                            x_sb[t, e] = make_x(t, e)

                ps_d = {}
                sq_sb = {}
                dc_sb = {}
                ps_t = {}

                def mm1(t):
                    ps_h = [psq.tile([P, TT], dt.float32, tag=f"psq{h}",
                                     name=f"psq{h}")
                            for h in range(HC)]
                    for e in range(EC):
                        for h in range(HC):
                            hsl = slice(h * P, (h + 1) * P)
                            nc.tensor.matmul(ps_h[h], wt_sb[e][:, hsl],
                                             x_sb[t, e],
                                             start=(e == 0), stop=(e == EC - 1))
                    return ps_h

                def dot(t):
                    ps_d[t] = psd.tile([O, TT], dt.float32, tag=f"psd{t}",
                                       name=f"psd{t}")
                    for e in range(EC):
                        nc.tensor.matmul(ps_d[t], a22_sb[:, e], x_sb[t, e],
                                         start=(e == 0), stop=False)

                def squares(t, ps_h):
                    for h in range(HC):
                        sq = work.tile([P, TT], dt.float32r, tag=f"sq{t}{h}",
                                       name=f"sq{t}{h}")
                        nc.scalar.activation(sq, ps_h[h], AF.Square)
                        sq_sb[t, h] = sq

                def norm(t):
                    for h in range(HC):
                        nc.tensor.matmul(ps_d[t], ones_sb, sq_sb[t, h],
                                         start=False, stop=(h == HC - 1))

                def dcopy(t):
                    dc = work.tile([O, TT], dt.float32, tag=f"dc{t}",
                                   name=f"dc{t}")
                    nc.scalar.copy(dc, ps_d[t])
                    dc_sb[t] = dc

                def transp(t):
                    ps_t[t] = pst.tile([P, NC_, O], dt.float32, tag=f"psT{t}",
                                       name=f"psT{t}")
                    for c in range(NC_):
                        nc.tensor.matmul(
                            ps_t[t][:, c], dc_sb[t][:, c * P:(c + 1) * P],
                            ident_sb[:O, :O], is_transpose=True,
                            start=True, stop=True)

                def softmax_tail(t):
                    tsl = slice(t * TT, (t + 1) * TT)
                    d2c = work.tile([P, NC_, O], dt.float32, tag=f"d2c{t}",
                                    name=f"d2c{t}")
                    nc.vector.tensor_tensor(
                        d2c, ps_t[t],
                        nrme_sb[:, None, :].to_broadcast((P, NC_, O)), ALU.add)
                    u_sb = work.tile([P, NC_, O], dt.float32, tag=f"u{t}",
                                     name=f"u{t}")
                    nc.scalar.activation(u_sb, d2c, AF.Ln)
                    s_sb = work.tile([P, NC_, O], dt.float32, tag=f"s{t}",
                                     name=f"s{t}")
                    nc.scalar.activation(s_sb, u_sb, AF.Exp, scale=0.5)
                    rs = work.tile([P, NC_, O], dt.float32, tag=f"rs{t}",
                                   name=f"rs{t}")
                    nc.vector.reciprocal(rs, s_sb)
                    dq = work.tile([P, NC_, O], dt.float32, tag=f"dq{t}",
                                   name=f"dq{t}")
                    nc.vector.tensor_tensor(dq, d2c, rs, ALU.mult)
                    s2 = work.tile([P, NC_, O], dt.float32, tag=f"s2{t}",
                                   name=f"s2{t}")
                    nc.vector.tensor_tensor(s2, s_sb, dq, ALU.add)
                    e_sb = work.tile([P, NC_, O], dt.float32, tag=f"e{t}",
                                     name=f"e{t}")
                    nc.scalar.activation(e_sb, s2, AF.Exp, scale=-0.5)
                    ssum = work.tile([P, NC_], dt.float32, tag=f"ssum{t}",
                                     name=f"ssum{t}")
                    nc.vector.reduce_sum(ssum, e_sb, axis=mybir.AxisListType.X)
                    rec = work.tile([P, NC_], dt.float32, tag=f"rec{t}",
                                    name=f"rec{t}")
                    nc.vector.reciprocal(rec, ssum)
                    p1 = work.tile([P, NC_, O], dt.float32, tag=f"p1{t}",
                                   name=f"p1{t}")
                    nc.vector.tensor_tensor(
                        p1, e_sb, rec[:, :, None].to_broadcast((P, NC_, O)),
                        ALU.mult)
                    msk = work.tile([P, NC_, O], dt.float32, tag=f"msk{t}",
                                    name=f"msk{t}")
                    nc.vector.tensor_scalar(msk, p1, THRESH, None, ALU.is_ge)
                    ot = work.tile([P, NC_, O], dt.float32, tag=f"ot{t}",
                                   name=f"ot{t}")
                    nc.vector.tensor_tensor(ot, p1, msk, ALU.mult)
                    nc.sync.dma_start(
                        out_d[tsl].rearrange("(c p) o -> p c o", p=P), ot)

                # schedule: tile-0 chain hides under tile-1's matmuls
                ph0 = mm1(0)
                dot(0)
                squares(0, ph0)
                norm(0)
                dcopy(0)
                ph1 = mm1(1)
                transp(0)
                softmax_tail(0)
                dot(1)
                squares(1, ph1)
                norm(1)
                dcopy(1)
                transp(1)
                softmax_tail(1)

    nc.compile()
    return nc


def _prep_inputs(word_output, W, b, op_emb):
    x = np.asarray(word_output, np.float32).reshape(TOK, E)
    xt = np.ascontiguousarray(x.T)                          # [E, TOK] fp32

    wt = np.ascontiguousarray(np.asarray(W, np.float32).T)  # [E, H] fp32

    ep = np.asarray(op_emb, np.float64) - np.asarray(b, np.float64)  # [O, H]
    A = (-2.0 * (ep @ np.asarray(W, np.float64))).T         # [E, O]
    a22 = np.ascontiguousarray(A.astype(np.float32))

    nrme = (ep * ep).sum(-1).astype(np.float32)             # [O]
    nrme = np.broadcast_to(nrme, (P, O)).copy()             # [P, O]
    ones32 = np.ones((P, O), np.float32)

    common = {"wt": wt, "a22": a22, "nrme": nrme, "ones32": ones32}
    in_maps = []
    for c in range(NCORES):
        tsl = slice(c * TPC, (c + 1) * TPC)
        m = dict(common)
        m["x"] = np.ascontiguousarray(xt[:, tsl])
        in_maps.append(m)
    return in_maps


def kernel(word_output, W, b, op_emb, _trace=False):
    if "nc" not in _CACHE:
        _CACHE["nc"] = _build()
    nc = _CACHE["nc"]
    in_maps = _prep_inputs(word_output, W, b, op_emb)
    try:
        res = bass_utils.run_bass_kernel_spmd(
            nc, in_maps, core_ids=list(range(NCORES)), trace=_trace)
    except ModuleNotFoundError:
        res = bass_utils.run_bass_kernel_spmd(
            nc, in_maps, core_ids=list(range(NCORES)), trace=False)
    out = np.concatenate([r["out"] for r in res.results], axis=0)
    _CACHE["last_results"] = res
    return out.reshape(B, S, O)


if __name__ == "__main__":
    rng = np.random.default_rng(0)
    wo = rng.standard_normal((B, S, E)).astype(np.float32)
    W_ = (rng.standard_normal((H, E)) / np.sqrt(E)).astype(np.float32)
    b_ = (rng.standard_normal(H) * 0.01).astype(np.float32)
    oe = rng.standard_normal((O, H)).astype(np.float32)
    out = kernel(wo, W_, b_, oe)
    x = wo.reshape(-1, E).astype(np.float64)
    proj = x @ W_.T.astype(np.float64) + b_
    diff = proj[:, None, :] - oe
    d2 = (diff * diff).sum(-1)
    dis = -np.sqrt(d2)
    exm = np.exp(dis - dis.max(-1, keepdims=True))
    prob = exm / exm.sum(-1, keepdims=True)
    ref = np.where(prob < THRESH, 0, prob).astype(np.float32).reshape(B, S, O)
    print("norm rel err:", np.linalg.norm(out - ref) / np.linalg.norm(ref))
    print("max abs err:", np.abs(out - ref).max())
